# revision 17
# baseline (speedup 1.0000x reference)
"""Bilinear RoI pooling, V4: overlap-tiled gather-matmul, slot-dealt cells.

The fp16 feature map lives in SBUF as per-(band, x-block) tiles

    T[v,b][p = yp*64 + xo, c] = feats[y = v + yp, x = 63*b + xo, c]

(v = 0..62 band = base row y0, yp in {0,1}, b = x-block 0..3 of width 63,
x-tile width 64 so x0+1 stays in-tile).  Every sample (one output pixel of
one RoI) is a 4-hot fp16 column against exactly ONE tile: weights at
partitions (xo, xo+1, 64+xo, 64+xo+1).  One matmul per (tile, channel
chunk) computes psum[c, s] = sum_k w_k[s] * feats[y_k, x_k, c] -- no
accumulation chains, no odd/even double pass (rows are stored twice
instead: ~4.2 MB/core).

x >= 252 ("tail") samples use a packed tile holding 16 band-pairs x 4
x-columns in the 128 partitions, so the whole tail is a handful of
matmuls.

Work distribution: the 63x4 (band, block) cells (large ones split into
<=1000-column pieces) are LPT-dealt to the 8 cores; each core packs its
pieces into shared schedule slots sorted by size, so per-slot quotas
(max over cores) give ONE static graph with ~2% padding.  The whole rhs
([128, ncol] fp16, 4-hot columns) is loaded once and stays resident.

PSUM is drained by BOTH the DVE and the Activation engine (alternating
256-column supers), cast fp32->fp16 into per-engine store rings, and
written out in 4-super batches.  Per-core HBM traffic ~37 MB
(4.3 F4 + 6.6 rhs + 26 stores) vs ~57 MB for V3."""

import hashlib
import heapq

import numpy as np

HH, WW = 7, 7
C, Hf, Wf = 512, 64, 256
NPY, NPX = Hf - 1, Wf - 1         # base grids: y0 in 0..62, x0 in 0..254
N_CORES = 8
B_TOTAL = 4096
S_CORE = B_TOTAL * HH * WW // N_CORES   # 25088 samples per core
XBW = 63                          # x-block width (blocks 0..3; x>=252 = tail)
NBLK = 4
SPLIT_MAX = 1000                  # max columns per schedule slot (cell piece)
SUP = 256                         # psum super columns
NRING = 16                        # store-ring buffers per cast engine
STB = 2                           # supers per store DMA batch
RSLAB_N = 8                       # rhs load slabs

_NC_CACHE = {}


def _build_nc(plan):
    import concourse.bacc as bacc
    import concourse.mybir as mybir

    quota = [int(q) for q in plan["quota"]]          # per-slot columns
    quota_tail = int(plan["quota_tail"])
    n_slots = len(quota)
    ncol = sum(quota) + quota_tail
    assert ncol % SUP == 0
    n_supers = ncol // SUP
    # cast owner per super: even -> DVE, odd -> ACT
    own = [s % 2 for s in range(n_supers)]
    dve_list = [s for s in range(n_supers) if own[s] == 0]
    act_list = [s for s in range(n_supers) if own[s] == 1]
    s2stream = {}
    for j, s in enumerate(dve_list):
        s2stream[s] = (0, j)
    for j, s in enumerate(act_list):
        s2stream[s] = (1, j)
    nb = [(len(dve_list) + STB - 1) // STB, (len(act_list) + STB - 1) // STB]
    # f4 load chunks: small leading chunks so the PE can start early
    fbounds = [0, 2, 4]
    while fbounds[-1] < n_slots:
        fbounds.append(min(n_slots, fbounds[-1] + 4))
    n_f4ch = len(fbounds) - 1
    slot_chunk = {}
    for k in range(n_f4ch):
        for l in range(fbounds[k], fbounds[k + 1]):
            slot_chunk[l] = k
    # rhs load slabs: small leading slab, then even 256-aligned splits
    rs = ((ncol - 1024 + (RSLAB_N - 2)) // (RSLAB_N - 1) + SUP - 1) // SUP * SUP
    rbounds = [0, 1024]
    while rbounds[-1] < ncol:
        rbounds.append(min(ncol, rbounds[-1] + rs))
    n_rslab = len(rbounds) - 1

    # ---- static matmul schedule: (slot|'T', q, a, b, super) ----
    # segment-major: all 4 channel chunks of a super-segment before the
    # next segment, so supers close strictly in column order (a slot may
    # span many supers; q-major would open block s%4 before closing s-4).
    sched = []
    col = 0
    for l in [*range(n_slots), "T"]:
        hi = ncol if l == "T" else col + quota[l]
        a = col
        while a < hi:
            s = a // SUP
            b = min(hi, (s + 1) * SUP)
            for q in range(4):
                sched.append((l, q, a, b, s))
            a = b
        col = hi
    first_touch = {}
    last_touch = {}
    for i, ins in enumerate(sched):
        s = ins[4]
        first_touch.setdefault(s, i)
        last_touch[s] = i
    assert set(first_touch) == set(range(n_supers)), "super coverage hole"
    lt = [last_touch[s] for s in range(n_supers)]
    assert lt == sorted(lt), "non-monotone super retirement"
    inc_at = {i: s for s, i in last_touch.items()}
    wait_at = {i: s for s, i in first_touch.items()}

    nc = bacc.Bacc("TRN2", debug=False)
    f16, f32 = mybir.dt.float16, mybir.dt.float32

    f4_d = nc.dram_tensor("f4", [128, n_slots * C], f16, kind="ExternalInput")
    tl_d = nc.dram_tensor("tl", [128, C], f16, kind="ExternalInput")
    rhs_d = nc.dram_tensor("rhs", [128, ncol], f16, kind="ExternalInput")
    out_d = [
        nc.dram_tensor(nm, [nbk, 128, STB * 1024], f16, kind="ExternalOutput")
        for nm, nbk in (("out_dve", nb[0]), ("out_act", nb[1]))
    ]

    f4 = nc.alloc_sbuf_tensor("f4_sb", [128, n_slots * C], f16)
    tl = nc.alloc_sbuf_tensor("tl_sb", [128, C], f16)
    rhs = nc.alloc_sbuf_tensor("rhs_sb", [128, ncol], f16)
    st = [
        nc.alloc_sbuf_tensor(f"st{e}", [128, NRING * 1024], f16) for e in range(2)
    ]
    ps = nc.alloc_psum_tensor("ps", [128, 4096], f32)

    f_sems = [nc.alloc_semaphore(f"f_sem{i}") for i in range(n_f4ch)]
    t_sem = nc.alloc_semaphore("t_sem")
    r_sems = [nc.alloc_semaphore(f"r_sem{i}") for i in range(n_rslab)]
    pe_sem = nc.alloc_semaphore("pe_sem")
    cast_sems = [nc.alloc_semaphore(f"cast_sem{e}") for e in range(2)]
    # one sem per store-ring position: same-position stores are serialized
    # by the ring-reuse cast gating, so thresholds are unambiguous even
    # with out-of-order DMA completions across positions.
    NPOS = NRING // STB
    st_sems = [
        [nc.alloc_semaphore(f"st_sem{e}_{p}") for p in range(NPOS)]
        for e in range(2)
    ]

    def cast_wait_for(engine, s):
        """Wait until super s is cast (psum block reusable)."""
        e, j = s2stream[s]
        engine.wait_ge(cast_sems[e], j + 1)

    def emit_cast(engine, e, s, j, copy):
        engine.wait_ge(pe_sem, s + 1)
        if j >= NRING:
            k_need = j // STB - NPOS          # store batch freeing this buf
            engine.wait_ge(st_sems[e][k_need % NPOS], 16 * (k_need // NPOS + 1))
        dst = st[e][:, (j % NRING) * 1024 : (j % NRING) * 1024 + 1024]
        src = ps[:, (s % 4) * 1024 : (s % 4) * 1024 + 1024]
        copy(dst, src).then_inc(cast_sems[e], 1)

    with nc.Block() as block:

        def load_f4(eng, k):
            c0, c1 = fbounds[k] * C, fbounds[k + 1] * C
            eng.dma_start(f4[:, c0:c1], f4_d[:, c0:c1]).then_inc(f_sems[k], 16)

        def load_rhs(eng, k):
            a, b = rbounds[k], rbounds[k + 1]
            eng.dma_start(rhs[:, a:b], rhs_d[:, a:b]).then_inc(r_sems[k], 16)

        @block.scalar
        def _(scalar):
            # PE-critical first loads on a fast HWDGE queue, then drain
            # odd supers; the late loads go through the idle gpsimd engine.
            load_f4(scalar, 0)
            load_rhs(scalar, 0)
            load_f4(scalar, 1)
            for j, s in enumerate(act_list):
                emit_cast(scalar, 1, s, j, scalar.copy)

        @block.gpsimd
        def _(gp):
            nch = max(n_f4ch, n_rslab)
            for k in range(2, nch):
                if k < n_f4ch:
                    load_f4(gp, k)
                if k < n_rslab:
                    load_rhs(gp, k)
                if k == 3:
                    gp.dma_start(tl[:, :], tl_d[:, :]).then_inc(t_sem, 16)

        @block.vector
        def _(vector):
            for j, s in enumerate(dve_list):
                emit_cast(vector, 0, s, j, vector.tensor_copy)

        @block.tensor
        def _(tensor):
            seen_slot = set()
            rmax = [0]
            for i, (l, q, a, b, s) in enumerate(sched):
                if l not in seen_slot:
                    seen_slot.add(l)
                    if l == "T":
                        tensor.wait_ge(t_sem, 16)
                    else:
                        tensor.wait_ge(f_sems[slot_chunk[l]], 16)
                while rmax[0] < n_rslab and rbounds[rmax[0]] < b:
                    tensor.wait_ge(r_sems[rmax[0]], 16)
                    rmax[0] += 1
                if i in wait_at and wait_at[i] >= 4:
                    cast_wait_for(tensor, wait_at[i] - 4)
                if l == "T":
                    lhsT = tl[:, 128 * q : 128 * (q + 1)]
                else:
                    lhsT = f4[:, l * C + 128 * q : l * C + 128 * (q + 1)]
                off = (s % 4) * 1024 + q * SUP + (a - s * SUP)
                mm = tensor.matmul(
                    ps[:, off : off + (b - a)],
                    lhsT,
                    rhs[:, a:b],
                    start=True,
                    stop=True,
                    skip_group_check=True,
                )
                if i in inc_at:
                    mm.then_inc(pe_sem, 1)

        @block.sync
        def _(sync):
            load_rhs(sync, 1)
            # all stores, both streams, ordered by global super time
            batches = []
            for e in range(2):
                n_e = len((dve_list, act_list)[e])
                for k in range(nb[e]):
                    last_s = ((dve_list, act_list)[e])[min(STB * (k + 1) - 1, n_e - 1)]
                    batches.append((last_s, e, k, min(STB * (k + 1), n_e)))
            batches.sort()
            for _, e, k, cth in batches:
                sync.wait_ge(cast_sems[e], cth)
                r0 = (STB * k % NRING) * 1024
                sync.dma_start(
                    out_d[e][k], st[e][:, r0 : r0 + STB * 1024]
                ).then_inc(st_sems[e][k % NPOS], 16)
            for e in range(2):
                for p in range(NPOS):
                    cnt = (nb[e] - p + NPOS - 1) // NPOS if nb[e] > p else 0
                    if cnt:
                        sync.wait_ge(st_sems[e][p], 16 * cnt)

    nc.compile()
    return nc


def _get_nc(plan):
    key = hashlib.sha256(
        np.asarray(plan["quota"], np.int64).tobytes()
        + np.int64(plan["quota_tail"]).tobytes()
    ).hexdigest()
    if key not in _NC_CACHE:
        _NC_CACHE[key] = _build_nc(plan)
    return _NC_CACHE[key]


def _host_prep(feats, boxes, img_height, img_width):
    """Per-sample base row (y0*255 + x0, clamped) and 4 slot weights
    (tl, tr, bl, br with validity and clamp-aggregation folded in),
    mirroring the reference math in f32."""
    B = boxes.shape[0]
    f32 = np.float32
    xc, yc, w, h = (boxes[:, k].astype(f32) for k in range(4))
    tx = np.linspace(-1.0, 1.0, WW, dtype=f32)
    ty = np.linspace(-1.0, 1.0, HH, dtype=f32)
    inv_w = f32(1.0) / f32(img_width - 1)
    inv_h = f32(1.0) / f32(img_height - 1)
    gx = (f32(2.0) * xc[:, None] - f32(img_width - 1)) * inv_w \
        + (w * inv_w)[:, None] * tx[None, :]
    gy = (f32(2.0) * yc[:, None] - f32(img_height - 1)) * inv_h \
        + (h * inv_h)[:, None] * ty[None, :]
    px = (gx + f32(1.0)) * f32(0.5) * f32(Wf - 1)   # (B, WW)
    py = (gy + f32(1.0)) * f32(0.5) * f32(Hf - 1)   # (B, HH)

    x0 = np.floor(px)
    y0 = np.floor(py)
    fx, fy = px - x0, py - y0
    x0i, y0i = x0.astype(np.int64), y0.astype(np.int64)
    x1i, y1i = x0i + 1, y0i + 1
    vx0 = ((x0i >= 0) & (x0i <= Wf - 1)).astype(f32)
    vx1 = ((x1i >= 0) & (x1i <= Wf - 1)).astype(f32)
    vy0 = ((y0i >= 0) & (y0i <= Hf - 1)).astype(f32)
    vy1 = ((y1i >= 0) & (y1i <= Hf - 1)).astype(f32)
    x0c = np.clip(x0i, 0, Wf - 1).astype(np.int32)
    x1c = np.clip(x1i, 0, Wf - 1).astype(np.int32)
    y0c = np.clip(y0i, 0, Hf - 1).astype(np.int32)
    y1c = np.clip(y1i, 0, Hf - 1).astype(np.int32)

    def by(a):
        return np.broadcast_to(a[:, :, None], (B, HH, WW))

    def bx(a):
        return np.broadcast_to(a[:, None, :], (B, HH, WW))

    base_y = np.clip(y0i, 0, NPY - 1)                 # (B, HH)
    base_x = np.clip(x0i, 0, NPX - 1)                 # (B, WW)
    rows = (by(base_y) * NPX + bx(base_x)).reshape(-1).astype(np.int32)

    wx0, wx1 = f32(1.0) - fx, fx
    wy0, wy1 = f32(1.0) - fy, fy
    wk = np.stack(
        [
            by(wy0 * vy0) * bx(wx0 * vx0),
            by(wy0 * vy0) * bx(wx1 * vx1),
            by(wy1 * vy1) * bx(wx0 * vx0),
            by(wy1 * vy1) * bx(wx1 * vx1),
        ],
        axis=-1,
    ).reshape(B * HH * WW, 4).astype(f32)
    dy = np.stack(
        [by(y0c - base_y), by(y0c - base_y), by(y1c - base_y), by(y1c - base_y)],
        axis=-1,
    ).reshape(B * HH * WW, 4)
    dx = np.stack(
        [bx(x0c - base_x), bx(x1c - base_x), bx(x0c - base_x), bx(x1c - base_x)],
        axis=-1,
    ).reshape(B * HH * WW, 4)
    slots = np.clip(dy, 0, 1) * 2 + np.clip(dx, 0, 1)
    wts = np.zeros((B * HH * WW, 4), f32)
    np.add.at(wts, (np.arange(B * HH * WW)[:, None], slots), wk)
    return rows, wts


def _prepare(feats, boxes, img_height, img_width):
    rows, wts = _host_prep(feats, boxes, img_height, img_width)
    n = rows.shape[0]
    y0 = (rows // NPX).astype(np.int64)          # 0..62
    x0 = (rows % NPX).astype(np.int64)           # 0..254
    blk = np.minimum(x0 // XBW, NBLK)            # 0..4 (4 = tail)
    is_tail = blk == NBLK

    # per-cell sample id lists (stable order)
    cell_of = y0 * (NBLK + 1) + blk
    order = np.argsort(cell_of, kind="stable")
    co = cell_of[order]
    starts = np.r_[0, np.flatnonzero(co[1:] != co[:-1]) + 1]
    uniq = co[starts]
    lens = np.diff(np.r_[starts, n])
    cell_ids = {int(u): order[s : s + L] for u, s, L in zip(uniq, starts, lens)}

    # split non-tail cells into pieces of <= SPLIT_MAX columns
    pieces = []                                   # (size, v, b, off)
    for v in range(NPY):
        for b in range(NBLK):
            ids = cell_ids.get(v * (NBLK + 1) + b)
            if ids is None:
                continue
            cnum = len(ids)
            k = -(-cnum // SPLIT_MAX)
            base, rem = cnum // k, cnum % k
            off = 0
            for j in range(k):
                sz = base + (1 if j < rem else 0)
                pieces.append((sz, v, b, off))
                off += sz
    pieces.sort(reverse=True)

    # LPT deal to cores; per-core lists stay size-sorted by re-sorting
    heap = [(0, 0, m) for m in range(N_CORES)]
    heapq.heapify(heap)
    percore = [[] for _ in range(N_CORES)]
    for p in pieces:
        tot, ns, m = heapq.heappop(heap)
        percore[m].append(p)
        heapq.heappush(heap, (tot + p[0], ns + 1, m))
    for m in range(N_CORES):
        percore[m].sort(reverse=True)
    n_slots = max(len(p) for p in percore)
    quota = np.zeros(n_slots, np.int64)
    for m in range(N_CORES):
        for l, p in enumerate(percore[m]):
            quota[l] = max(quota[l], p[0])

    # tail cells: snake-deal by size (<= 16 per core)
    tcells = sorted(
        (
            (len(cell_ids[v * (NBLK + 1) + NBLK]), v)
            for v in range(NPY)
            if v * (NBLK + 1) + NBLK in cell_ids
        ),
        reverse=True,
    )
    tcore = [[] for _ in range(N_CORES)]
    ttot = np.zeros(N_CORES, np.int64)
    for i, c in enumerate(tcells):
        r, m = divmod(i, N_CORES)
        m = m if r % 2 == 0 else N_CORES - 1 - m
        tcore[m].append(c)
        ttot[m] += c[0]
    assert max(len(t) for t in tcore) <= 16, "tail tile overflow"
    qt = int(ttot.max())
    ncol = int(quota.sum()) + qt
    qt += (-ncol) % SUP
    ncol += (-ncol) % SUP
    plan = {"quota": quota, "quota_tail": qt, "ncol": ncol}

    slot_start = np.zeros(n_slots + 1, np.int64)
    np.cumsum(quota, out=slot_start[1:])
    tail_start = int(slot_start[n_slots])

    ftab = feats.astype(np.float16)               # (C, Hf, Wf)
    yp_ = np.arange(128) // 64
    xo_ = np.arange(128) % 64

    in_maps, colmaps = [], []
    for m in range(N_CORES):
        f4_dat = np.zeros((128, n_slots * C), np.float16)
        rhs = np.zeros((128, ncol), np.float16)
        colmap = np.full(ncol, -1, np.int64)
        for l, (sz, v, b, off) in enumerate(percore[m]):
            ids = cell_ids[v * (NBLK + 1) + b][off : off + sz]
            # tile: [p = yp*64 + xo, c] = feats[v + yp, 63*b + xo, c]
            f4_dat[:, l * C : (l + 1) * C] = ftab[
                :, v + yp_, XBW * b + xo_
            ].T
            cols = slot_start[l] + np.arange(sz)
            colmap[cols] = ids
            xo = x0[ids] - XBW * b
            w4 = wts[ids]
            rhs[xo, cols] = w4[:, 0]
            rhs[xo + 1, cols] = w4[:, 1]
            rhs[64 + xo, cols] = w4[:, 2]
            rhs[64 + xo + 1, cols] = w4[:, 3]
        # tail tile: [p = u*8 + yp*4 + xoff, c] = feats[v_u + yp, 252 + xoff, c]
        tl_dat = np.zeros((128, C), np.float16)
        tcol = tail_start
        for u, (sz, v) in enumerate(tcore[m]):
            p_ = np.arange(8)
            tl_dat[u * 8 + p_] = ftab[
                :, v + p_ // 4, np.minimum(NBLK * XBW + p_ % 4, Wf - 1)
            ].T
            ids = cell_ids[v * (NBLK + 1) + NBLK]
            cols = tcol + np.arange(sz)
            tcol += sz
            colmap[cols] = ids
            xoff = x0[ids] - NBLK * XBW
            w4 = wts[ids]
            rhs[u * 8 + xoff, cols] = w4[:, 0]
            rhs[u * 8 + xoff + 1, cols] = w4[:, 1]
            rhs[u * 8 + 4 + xoff, cols] = w4[:, 2]
            rhs[u * 8 + 4 + xoff + 1, cols] = w4[:, 3]
        in_maps.append(
            {
                "f4": f4_dat,
                "tl": tl_dat,
                "rhs": np.ascontiguousarray(rhs),
            }
        )
        colmaps.append(colmap)

    return plan, in_maps, colmaps


def kernel(**inputs):
    from concourse.bass_utils import run_bass_kernel_spmd

    feats = np.asarray(inputs["feats"], dtype=np.float32)
    boxes = np.asarray(inputs["boxes"], dtype=np.float32)
    img_height = int(np.asarray(inputs["img_height"]))
    img_width = int(np.asarray(inputs["img_width"]))

    plan, in_maps, colmaps = _prepare(feats, boxes, img_height, img_width)
    nc = _get_nc(plan)
    res = run_bass_kernel_spmd(nc, in_maps, core_ids=list(range(N_CORES)))

    ncol = plan["ncol"]
    n_supers = ncol // SUP
    out_all = np.empty((C, B_TOTAL * HH * WW), np.float32)
    for m, r in enumerate(res.results):
        full = np.empty((C, ncol), np.float32)
        for e, nm in enumerate(("out_dve", "out_act")):
            a = r[nm]                              # (nb, 128, 4*1024) f16
            slist = [s for s in range(n_supers) if s % 2 == e]
            nbk = a.shape[0]
            # a[k, p, j*1024 + q*256 + r] -> [q*128 + p, (k*STB + j)*SUP + r]
            x = (
                a.reshape(nbk, 128, STB, 4, SUP)
                .transpose(3, 1, 0, 2, 4)
                .reshape(C, nbk * STB * SUP)
            )
            for j, s in enumerate(slist):
                full[:, s * SUP : (s + 1) * SUP] = x[
                    :, j * SUP : (j + 1) * SUP
                ].astype(np.float32)
        cm = colmaps[m]
        valid = cm >= 0
        out_all[:, cm[valid]] = full[:, valid]
    out = out_all.T.reshape(B_TOTAL, HH * WW, C).transpose(0, 2, 1)
    return np.ascontiguousarray(out.reshape(B_TOTAL, C, HH, WW)).astype(np.float32)
